# revision 24
# baseline (speedup 1.0000x reference)
"""Trainium2 Bass kernel for nn_CausalConvolution (dense_cnn).

Reference computation (B=4, S=4096, H=2048, CIN=COUT=4096, K=4, G=8):
    h   = x @ W_in.T + b_in                       # [B,S,CIN]
    y   = silu(causal_grouped_conv1d(h) + conv_b) # [B,S,COUT], groups=8, k=4
    out = y @ W_out.T + b_out                     # [B,S,H]

Sharding: one conv group per NeuronCore (G = 8 = n_cores).
Core g computes channels [g*512, (g+1)*512) of h (column-parallel W_in),
its conv group (512 in / 512 out channels), and a row-parallel partial of
the output projection. Host sums the 8 partials (stored bf16) and adds
b_out. No cross-core communication on device.

All matmuls run with the contraction dim on SBUF partitions in a
"transposed" [channel, time] layout, bf16 with fp32 PSUM accumulation —
except the last N8 (=4) of stage 1's 16 contraction chunks, which run as
fp8e4 DoubleRow pairs (2 chunks per matmul, 2x PE throughput). To let
fp8 and bf16 products share one PSUM accumulation, ALL stage-1 operands
are pre-scaled by powers of two on the host (x*8, W_in*256; exact in
bf16), and the stage-1 activation applies 1/2048. rel_err budget: e4m3
on 4/16 chunks costs ~1.7e-2 of the 2e-2 allowance (measured in sim).

Schedule notes (from perfetto analysis of prior revisions):
- PE is the bottleneck: 5632 bf16 N=512 matmuls + 256 fp8 DoubleRow
  matmuls/core ~= 1.26 ms streaming floor. Everything else hides behind
  it or dies trying.
- DMA descriptors drain roughly FIFO per hwdge ring with bandwidth
  shared across all in-flight descriptors, so ISSUE ORDER is the
  scheduling tool: the sync ring carries x tiles (+ half the output
  stores), the scalar ring carries all weights in deadline order
  (w_in c0, c1, bias, c2, c3, conv, out) + the other half of stores.
- The PE warmup scratch must come from the persistent weight pool: a
  scratch in its own pool gets its SBUF reused for w_in, and the WAR
  dependency then blocks the critical first weight DMA until warmup
  ends (cost ~10 us, found the hard way).
- Stage 1 runs 3 tiles ahead of stage 2, stage 3 one tile behind
  stage 2, so the conv/out weights' arrival deadlines (~30/~45 us)
  clear while the PE chews through x-only work at the start.
"""

import numpy as np
import ml_dtypes

# Problem constants (hardcoded per the harness contract).
B, S, H = 4, 4096, 2048
CIN = COUT = 4096
KT = 4          # conv taps
G = 8           # conv groups == number of cores
CG = CIN // G   # 512 channels per group/core
T = B * S       # 16384 flattened time steps
NCORES = 8

HK = H // 128       # 16 contraction chunks for stage 1
N8 = 4              # stage-1 chunks done in fp8 DoubleRow (must be even)
HK16 = HK - N8      # stage-1 chunks done in bf16
CT = CG // 128      # 4 chunks of the per-core channel dim
TTILE = 512         # time-tile (N of every matmul)
NH = H // TTILE     # 4 output-column chunks of stage 3

SCALE_X = 8.0       # stage-1 operand pre-scales (powers of two, exact in bf16)
SCALE_W = 256.0
SCALE_INV = 1.0 / (SCALE_X * SCALE_W)

D1 = 3              # stage2 runs D1 tiles behind stage1
D2 = 1              # stage3 runs D2 tiles behind stage2

_BF16 = ml_dtypes.bfloat16
_F8 = ml_dtypes.float8_e4m3  # IEEE e4m3, max +-240 == TRN FP8_EXP4

_CACHE = {}

# test.py introspection: the most recent BassKernelResults from a run.
LAST_RESULTS = None


def _build_nc():
    import concourse.bass as bass
    import concourse.mybir as mybir
    import concourse.tile as tile
    from concourse import bacc

    dt = mybir.dt
    AF = mybir.ActivationFunctionType
    DR = mybir.MatmulPerfMode.DoubleRow

    nc = bacc.Bacc(
        "TRN2", target_bir_lowering=False, debug=False, num_devices=NCORES
    )

    xT16 = nc.dram_tensor("xT16", [128, HK16, T], dt.bfloat16, kind="ExternalInput")
    xT8 = nc.dram_tensor("xT8", [128, N8, T], dt.float8e4, kind="ExternalInput")
    w_in16 = nc.dram_tensor(
        "w_in16", [128, CT, HK16, 128], dt.bfloat16, kind="ExternalInput"
    )
    w_in8 = nc.dram_tensor(
        "w_in8", [128, CT, N8, 128], dt.float8e4, kind="ExternalInput"
    )
    cw = nc.dram_tensor("cw", [128, KT, CT, CG], dt.bfloat16, kind="ExternalInput")
    wo = nc.dram_tensor("wo", [128, CT, H], dt.bfloat16, kind="ExternalInput")
    b_in = nc.dram_tensor("b_in", [128, CT], dt.float32, kind="ExternalInput")
    cb = nc.dram_tensor("cb", [128, CT], dt.float32, kind="ExternalInput")
    # Per-core partials are summed on the host in fp32; storing them in
    # bf16 halves the store traffic and costs ~2e-4 extra rel err.
    out = nc.dram_tensor("out", [T, H], dt.bfloat16, kind="ExternalOutput")

    n_tt = S // TTILE  # time tiles per batch

    with tile.TileContext(nc) as tc:
        # Few pools: the kernel-exit barrier/drain chain costs ~1 us per
        # pool, so stream tiles share one pool via per-tag buf counts.
        with (
            tc.tile_pool(name="weights", bufs=1) as wpool,
            tc.tile_pool(name="stream", bufs=1) as spool,
            tc.tile_pool(name="psum", bufs=1, space="PSUM") as pspool,
        ):
            xpool = x8pool = hpool = ypool = opool = spool
            ps1pool = ps2pool = ps3pool = pspool
            # PE warmup: dep-free matmuls on scratch run while the first
            # weight/x DMAs are in flight, so HAM un-throttles (K=8/8)
            # before the real matmul stream begins. The scratch lives in
            # the persistent pool — see module docstring.
            scratch = wpool.tile([128, 640], dt.bfloat16)
            nc.vector.memset(scratch[:], 0.0)
            wps = ps3pool.tile([128, TTILE], dt.float32, tag="ps3", bufs=3)
            for _ in range(4):
                nc.tensor.matmul(
                    wps[:], scratch[:, 0:128], scratch[:, 128:640],
                    start=True, stop=True,
                )

            # ---- startup DMAs: ring order == drain order == priority ----
            # The startup is HBM-bandwidth-bound (stage 1 runs D1 tiles
            # ahead, consuming x at ~4x the steady rate), so descriptors
            # are ordered by consumption deadline. Weights go on the sync
            # ring; the first x tile on the scalar ring, which must be
            # drained of DMA issues before the first ACTIVATE needs it
            # (DMA backpressure on the issuing queue blocks later queue
            # entries). cw/wo are emitted mid-loop (after stage1(3)) so
            # they ride behind xt(1..3) in the sync ring.
            xt_first = xpool.tile([128, HK16, TTILE], dt.bfloat16, tag="xt", bufs=4)
            x8_first = x8pool.tile([128, N8, TTILE], dt.float8e4, tag="xt8", bufs=4)
            nc.scalar.dma_start(x8_first[:], xT8[:, :, 0:TTILE])
            for q in (0, 1, 3, 5):
                nc.scalar.dma_start(
                    xt_first[:, 2 * q : 2 * q + 2, :],
                    xT16[:, 2 * q : 2 * q + 2, 0:TTILE],
                )
            w16_sb = wpool.tile([128, CT, HK16, 128], dt.bfloat16)
            w8_sb = wpool.tile([128, CT, N8, 128], dt.float8e4)
            bin_sb = wpool.tile([128, CT], dt.float32)
            cb_sb = wpool.tile([128, CT], dt.float32)
            cw_sb = wpool.tile([128, KT, CT, CG], dt.bfloat16)
            wo_sb = wpool.tile([128, CT, H], dt.bfloat16)
            nc.sync.dma_start(w8_sb[:], w_in8[:])
            for half in range(2):
                nc.sync.dma_start(
                    w16_sb[:, 0, 6 * half : 6 * half + 6, :],
                    w_in16[:, 0, 6 * half : 6 * half + 6, :],
                )
            for q in (2, 4):
                nc.sync.dma_start(
                    xt_first[:, 2 * q : 2 * q + 2, :],
                    xT16[:, 2 * q : 2 * q + 2, 0:TTILE],
                )
            nc.sync.dma_start(w16_sb[:, 1], w_in16[:, 1])
            nc.sync.dma_start(w16_sb[:, 2], w_in16[:, 2])
            nc.sync.dma_start(w16_sb[:, 3], w_in16[:, 3])
            nc.sync.dma_start(bin_sb[:], b_in[:])
            nc.sync.dma_start(cb_sb[:], cb[:])

            tiles = [(b, tt) for b in range(B) for tt in range(n_tt)]
            hts = {}   # batch -> hT tile
            yts = {}   # (b, tt) -> y tile

            def stage1(b, tt):
                t0 = tt * TTILE
                tg = b * S + t0
                if tt == 0:
                    # h^T for this batch: [c, t] with a 3-column zero halo
                    # in front so causal taps at batch start read zeros.
                    hts[b] = hpool.tile(
                        [128, CT, KT - 1 + S], dt.bfloat16, tag="hT", name="hT",
                        bufs=2,
                    )
                    nc.vector.memset(hts[b][:, :, 0 : KT - 1], 0.0)
                hT = hts[b]
                if b == 0 and tt == 0:
                    xt, x8t = xt_first, x8_first
                else:
                    xt = xpool.tile([128, HK16, TTILE], dt.bfloat16, tag="xt", bufs=4)
                    x8t = x8pool.tile([128, N8, TTILE], dt.float8e4, tag="xt8", bufs=4)
                    nc.sync.dma_start(x8t[:], xT8[:, :, tg : tg + TTILE])
                    nc.sync.dma_start(xt[:, 0:6, :], xT16[:, 0:6, tg : tg + TTILE])
                    nc.sync.dma_start(xt[:, 6:12, :], xT16[:, 6:12, tg : tg + TTILE])
                def emit_dr(ps, c):
                    # fp8 DoubleRow pairs lead each accumulation group:
                    # they only need w8/x8 (0.5 MB), so on the first tile
                    # they give the PE real work during the bandwidth
                    # crunch while the bf16 x/weights stream in.
                    for j in range(N8 // 2):
                        nc.tensor.matmul(
                            ps[:],
                            w8_sb[:, c, 2 * j : 2 * j + 2, :],
                            x8t[:, 2 * j : 2 * j + 2, :],
                            start=(j == 0),
                            stop=False,
                            perf_mode=DR,
                        )

                def emit_bf16_act(ps, c):
                    for hk in range(HK16):
                        nc.tensor.matmul(
                            ps[:],
                            w16_sb[:, c, hk, :],
                            xt[:, hk, :],
                            start=False,
                            stop=(hk == HK16 - 1),
                        )
                    nc.scalar.activation(
                        hT[:, c, KT - 1 + t0 : KT - 1 + t0 + TTILE],
                        ps[:],
                        AF.Identity,
                        bias=bin_sb[:, c : c + 1],
                        scale=SCALE_INV,
                    )

                if b == 0 and tt == 0:
                    # Head tile: open 3 DR groups back-to-back across PSUM
                    # banks before any bf16 work (see emit_dr comment).
                    pss = []
                    for c in range(3):
                        ps = ps1pool.tile([128, TTILE], dt.float32, tag="ps1", bufs=3)
                        emit_dr(ps, c)
                        pss.append(ps)
                    for c in range(3):
                        emit_bf16_act(pss[c], c)
                    ps = ps1pool.tile([128, TTILE], dt.float32, tag="ps1", bufs=3)
                    emit_dr(ps, 3)
                    emit_bf16_act(ps, 3)
                else:
                    for c in range(CT):
                        ps = ps1pool.tile([128, TTILE], dt.float32, tag="ps1", bufs=3)
                        emit_dr(ps, c)
                        emit_bf16_act(ps, c)

            def stage2(b, tt):
                t0 = tt * TTILE
                hT = hts[b]
                # causal grouped conv as 16 accumulated matmuls per chunk
                yt = ypool.tile([128, CT, TTILE], dt.bfloat16, tag="yt", bufs=3)
                yts[(b, tt)] = yt
                for o in range(CT):
                    ps = ps2pool.tile([128, TTILE], dt.float32, tag="ps2", bufs=2)
                    n_acc = KT * CT
                    acc = 0
                    for ik in range(CT):
                        for k in range(KT):
                            nc.tensor.matmul(
                                ps[:],
                                cw_sb[:, k, ik, o * 128 : (o + 1) * 128],
                                hT[:, ik, t0 + k : t0 + k + TTILE],
                                start=(acc == 0),
                                stop=(acc == n_acc - 1),
                            )
                            acc += 1
                    nc.scalar.activation(
                        yt[:, o, :],
                        ps[:],
                        AF.Silu,
                        bias=cb_sb[:, o : o + 1],
                    )

            def stage3(b, tt):
                tg = b * S + tt * TTILE
                yt = yts.pop((b, tt))
                for ss in range(TTILE // 128):
                    ot = opool.tile([128, H], dt.bfloat16, tag="ot", bufs=4)
                    for nh in range(NH):
                        ps = ps3pool.tile([128, TTILE], dt.float32, tag="ps3", bufs=3)
                        for oo in range(CT):
                            nc.tensor.matmul(
                                ps[:],
                                yt[:, oo, ss * 128 : (ss + 1) * 128],
                                wo_sb[:, oo, nh * TTILE : (nh + 1) * TTILE],
                                start=(oo == 0),
                                stop=(oo == CT - 1),
                            )
                        nc.vector.tensor_copy(
                            ot[:, nh * TTILE : (nh + 1) * TTILE], ps[:]
                        )
                    row = tg + ss * 128
                    # Two half-row stores, one per hwdge ring.
                    nc.sync.dma_start(
                        out[row : row + 128, 0 : H // 2], ot[:, 0 : H // 2]
                    )
                    nc.scalar.dma_start(
                        out[row : row + 128, H // 2 : H], ot[:, H // 2 : H]
                    )

            n = len(tiles)
            for i in range(n):
                stage1(*tiles[i])
                if i == D1:
                    # Bulk weights enter the sync ring here: behind
                    # xt(1..D1) (earlier deadlines), ahead of xt(D1+1...).
                    for k in range(KT):
                        nc.sync.dma_start(cw_sb[:, k], cw[:, k])
                    for oo in range(CT):
                        nc.sync.dma_start(wo_sb[:, oo], wo[:, oo])
                if i >= D1:
                    stage2(*tiles[i - D1])
                if i >= D1 + D2:
                    stage3(*tiles[i - D1 - D2])
            for i in range(n - D1, n):
                stage2(*tiles[i])
                stage3(*tiles[i - D2])
            for i in range(n - D2, n):
                stage3(*tiles[i])

    nc.compile()
    return nc


def _prep_inputs(x, W_in, b_in, conv_w, conv_b, W_out):
    """Host-side shard + transpose + cast. Returns in_maps for 8 cores."""
    x = np.asarray(x, dtype=np.float32)
    xr = x.reshape(T, HK, 128).transpose(2, 1, 0)  # [128, HK, T]
    xr16 = np.ascontiguousarray(xr[:, :HK16, :] * SCALE_X).astype(_BF16)
    xr8 = np.ascontiguousarray(
        np.clip(xr[:, HK16:, :] * SCALE_X, -240, 240)
    ).astype(_F8)

    in_maps = []
    for g in range(NCORES):
        c0 = g * CG
        w_in_g = (
            np.asarray(W_in[c0 : c0 + CG, :])
            .reshape(CT, 128, HK, 128)
            .transpose(3, 0, 2, 1)
        ) * SCALE_W  # [128, CT, HK, 128]: (hi, cc, hk, ci)
        w16_g = np.ascontiguousarray(w_in_g[:, :, :HK16, :]).astype(_BF16)
        w8_g = np.ascontiguousarray(
            np.clip(w_in_g[:, :, HK16:, :], -240, 240)
        ).astype(_F8)
        cw_g = np.ascontiguousarray(
            np.asarray(conv_w[c0 : c0 + CG, :, :])
            .reshape(CG, CT, 128, KT)
            .transpose(2, 3, 1, 0)
            .astype(_BF16)
        )  # [128, KT, CT, CG]: (ii, k, io, o) = conv_w[c0+o, io*128+ii, k]
        wo_g = np.ascontiguousarray(
            np.asarray(W_out[:, c0 : c0 + CG])
            .reshape(H, CT, 128)
            .transpose(2, 1, 0)
            .astype(_BF16)
        )  # [128, CT, H]: (oi, oo, h) = W_out[h, c0+oo*128+oi]
        bin_g = np.ascontiguousarray(
            np.asarray(b_in[c0 : c0 + CG], dtype=np.float32).reshape(CT, 128).T
        )  # [128, CT]
        cb_g = np.ascontiguousarray(
            np.asarray(conv_b[c0 : c0 + CG], dtype=np.float32).reshape(CT, 128).T
        )
        in_maps.append(
            {
                "xT16": xr16,
                "xT8": xr8,
                "w_in16": w16_g,
                "w_in8": w8_g,
                "cw": cw_g,
                "wo": wo_g,
                "b_in": bin_g,
                "cb": cb_g,
            }
        )
    return in_maps


def kernel(x, W_in, b_in, conv_w, conv_b, W_out, b_out):
    global LAST_RESULTS
    from concourse import bass_utils

    if "nc" not in _CACHE:
        _CACHE["nc"] = _build_nc()
    nc = _CACHE["nc"]

    in_maps = _prep_inputs(x, W_in, b_in, conv_w, conv_b, W_out)

    res = bass_utils.run_bass_kernel_spmd(
        nc, in_maps, core_ids=list(range(NCORES))
    )
    LAST_RESULTS = res

    acc = np.asarray(res.results[0]["out"]).astype(np.float32)
    for r in res.results[1:]:
        acc += np.asarray(r["out"]).astype(np.float32)
    acc += np.asarray(b_out, dtype=np.float32)[None, :]
    return acc.reshape(B, S, H)
